# revision 26
# baseline (speedup 1.0000x reference)
"""Trainium2 Bass kernel for the PlanLoss (dist + collision CIoU) problem.

Full inputs in, full (scalar) output out. Sharded across 8 NeuronCores as
a 4 (N-blocks) x 2 (A-halves) grid:
  core c: n in [128*(c//2), 128*(c//2)+128), a in [64*(c%2), 64*(c%2)+64)

Device layout: partition = n (128), free = (k, t, a_chunk), t-major with
unit stride on a so per-(n,a[,k]) broadcast operands keep innermost
stride 1 and fp16 tensor_tensor ops hit the DVE 2x perf mode. The K=3
dimension is merged into single wide ops wherever no per-k immediate is
needed (per-k immediates survive only in the inter/c2 fused custom ops
and the weighted mask-reduce).

Math reformulation (verified vs reference, fp32 ~3e-7, fp16 ~2e-5):
  iw_k = relu(min(Uw, Gw_k - tau/2) + tau/2),  Uw = sw0 - |dx|
  cw_k = max(Vw, Hw_k - tau/2) + tau/2,        Vw = sw0 + |dx|
  Gw_k = min(w1, w2+tau), Hw_k = max(w1, w2+tau)
  ciou = iou - rho2/c2 - v^2/(1+eps+v-iou),  iou = inter/(S_k-inter)
Reciprocals on ACT via in-place exp(-ln(x)) (act-table sets pinned);
add/sub/mult wide ops go to GPSIMD, min/max and fused customs to DVE.

Per-core output [128, 3]: (sum_{a,k,t} w_k*ciou*validf, sum_t validf,
sum_t d*validf). Host divides by denom and averages (the unshard step).
"""

import contextlib

import numpy as np

import concourse.bass as bass
import concourse.mybir as mybir
import concourse.tile as tile
from concourse import bacc, dve_ops
from concourse.dve_spec import (
    C0,
    C1,
    Spec,
    Src0,
    Src1,
    Zero,
    lower,
    maxx,
    minn,
    sq,
    _has_src1,
)
from concourse.dve_uop import DveOpSpec

F32 = mybir.dt.float32
F16 = mybir.dt.float16
ALU = mybir.AluOpType
ACTF = mybir.ActivationFunctionType

EPS = 1e-7
TAU = (0.0, 0.5, 1.0)
WK = (1.0, 0.4, 0.1)
N, A, T, K = 512, 128, 64, 3
NL, AL = 128, 64          # per-core n rows / a columns
ACH = 16                  # a-chunk size
NCHUNK = AL // ACH
TWO_OVER_PI = float(2.0 / np.pi)


# --------------------------------------------------------------------------
# custom DVE ops (registered into concourse.dve_ops at import time)
# --------------------------------------------------------------------------

def _register(name, spec, subdim=False):
    for op in dve_ops.OPS:
        if op.name == name:
            return op
    row = dve_ops._CUSTOM_DVE_ROW_BASE + len(dve_ops.OPS)
    assert row < 0x20, "out of custom-DVE rows"
    dve_ops._SUB_OPCODE_FOR_NAME[name] = row
    shas = {}
    for ver in ("v3", "v4"):
        try:
            s = DveOpSpec(
                name=name, opcode=row, uops=lower(spec, ver=ver),
                rd1_en=_has_src1(spec),
            )
            shas[ver] = s.sha(ver)
        except Exception:
            pass
    op = dve_ops.DveOp(name, spec, subdim=subdim, uops_sha=shas)
    dve_ops.OPS.append(op)
    dve_ops.CUSTOM_DVE_SPECS[name] = spec
    return op


def _np32(f):
    return lambda *a: f(*[np.asarray(x, np.float32) if i < 2 else x
                          for i, x in enumerate(a)]).astype(np.float32)


# out = in0^2 + in1^2 + s0
OP_SQ_ADD = _register(
    "ANT_PL_SQ_ADD",
    Spec(body=sq(Src0) + sq(Src1) + C0,
         reference=_np32(lambda in0, in1, s0, s1, imm2: in0 * in0 + in1 * in1 + s0)),
)
# out = relu(in0 + s0) * relu(in1 + s0)
OP_RELU_BIAS_MUL = _register(
    "ANT_PL_RELU_B_MUL",
    Spec(body=maxx(Src0 + C0, Zero) * maxx(Src1 + C0, Zero),
         reference=_np32(lambda in0, in1, s0, s1, imm2:
                         np.maximum(in0 + s0, 0) * np.maximum(in1 + s0, 0))),
)
# out = (in0 + s0)^2 + (in1 + s0)^2 + s1
OP_SQB_ADD = _register(
    "ANT_PL_SQB_ADD",
    Spec(body=sq(Src0 + C0) + sq(Src1 + C0) + C1,
         reference=_np32(lambda in0, in1, s0, s1, imm2:
                         (in0 + s0) ** 2 + (in1 + s0) ** 2 + s1)),
)
# out = (in0 * s0)^2
OP_SQ1 = _register(
    "ANT_PL_SQ1",
    Spec(body=sq(Src0 * C0),
         reference=_np32(lambda in0, in1, s0, s1, imm2: (in0 * s0) ** 2)),
)


def _pin_act_sets():
    """Make Exp/Ln resolve uniquely to natural_log_exp_and_others and
    Arctan to sigmoid_and_others, so the act-table-load inserter doesn't
    ping-pong between per-function sets (each reload costs ~2.7us)."""
    from concourse.hw_specs import get_activation_tables
    tables = get_activation_tables("gen3")       # cached dict; mutate in place
    for name, fns in tables.items():
        if name != "natural_log_exp_and_others":
            fns.discard(ACTF.Exp)
            fns.discard(ACTF.Ln)
        if name != "sigmoid_and_others":
            fns.discard(ACTF.Arctan)


# --------------------------------------------------------------------------
# bass program
# --------------------------------------------------------------------------

def _build_nc(repeat=1):
    _pin_act_sets()
    nc = bacc.Bacc(trn_type="TRN2", num_devices=8, enable_asserts=False)

    PLANE = NCHUNK * T * ACH
    mamx = nc.dram_tensor("mamx", [NL, PLANE], F16, kind="ExternalInput")
    mamy = nc.dram_tensor("mamy", [NL, PLANE], F16, kind="ExternalInput")
    msz = nc.dram_tensor("msz", [NL, AL * 2], F32, kind="ExternalInput")
    ego = nc.dram_tensor("ego", [NL, 2], F32, kind="ExternalInput")
    pred = nc.dram_tensor("pred", [NL, T * 2], F32, kind="ExternalInput")
    targ = nc.dram_tensor("targ", [NL, T * 2], F32, kind="ExternalInput")
    out = nc.dram_tensor("out", [NL, 3], F32, kind="ExternalOutput")

    v = nc.vector
    g = nc.gpsimd
    sc = nc.scalar

    with tile.TileContext(nc) as tc:
        rep_ctx = tc.For_i(0, repeat, 1) if repeat > 1 else contextlib.nullcontext()
        with (
            tc.tile_pool(name="io", bufs=2) as io,
            tc.tile_pool(name="small", bufs=1) as small,
            tc.tile_pool(name="kind", bufs=2) as kind,
            tc.tile_pool(name="mg", bufs=1) as mg,      # merged [NL,K,T,ACH] ring
            tc.tile_pool(name="msc", bufs=3) as msc,
            rep_ctx,
        ):
            FT = [NL, T, ACH]          # per-k slice shape
            FK = [NL, K, T, ACH]       # K-merged shape

            # ---- small loads -------------------------------------------------
            ego_t = small.tile([NL, 2], F32, tag="ego", name="ego")
            nc.sync.dma_start(ego_t[:], ego[:])
            pred_t = small.tile([NL, T, 2], F32, tag="pred", name="pred")
            nc.sync.dma_start(pred_t[:], pred[:].rearrange("p (t c) -> p t c", c=2))
            targ_t = small.tile([NL, T, 2], F32, tag="targ", name="targ")
            nc.sync.dma_start(targ_t[:], targ[:].rearrange("p (t c) -> p t c", c=2))
            msz_t = small.tile([NL, AL, 2], F32, tag="msz", name="msz")
            nc.sync.dma_start(msz_t[:], msz[:].rearrange("p (a c) -> p a c", c=2))

            w1 = ego_t[:, 0:1]
            h1 = ego_t[:, 1:2]

            # ---- per-partition (n) scalars ----------------------------------
            w1h = small.tile([NL, 1], F32, tag="w1h", name="w1h")
            v.tensor_scalar_mul(w1h[:], w1, 0.5)
            h1h = small.tile([NL, 1], F32, tag="h1h", name="h1h")
            v.tensor_scalar_mul(h1h[:], h1, 0.5)
            area1 = small.tile([NL, 1], F32, tag="area1", name="area1")
            v.tensor_tensor(area1[:], w1, h1, op=ALU.mult)
            h1e = small.tile([NL, 1], F32, tag="h1e", name="h1e")
            v.tensor_scalar_add(h1e[:], h1, EPS)
            r_h1 = small.tile([NL, 1], F32, tag="r_h1", name="r_h1")
            v.reciprocal_approx_fast(r_h1[:], h1e[:])
            q1 = small.tile([NL, 1], F32, tag="q1", name="q1")
            v.tensor_tensor(q1[:], w1, r_h1[:], op=ALU.mult)
            at1 = small.tile([NL, 1], F32, tag="at1", name="at1")
            sc.activation(at1[:], q1[:], ACTF.Arctan)

            # ---- valid mask, denom, dist ------------------------------------
            vx = small.tile([NL, T], F32, tag="vx", name="vx")
            v.tensor_scalar(vx[:], targ_t[:, :, 0:1], -999.0, None,
                            op0=ALU.not_equal)
            vy = small.tile([NL, T], F32, tag="vy", name="vy")
            v.tensor_scalar(vy[:], targ_t[:, :, 1:2], -999.0, None,
                            op0=ALU.not_equal)
            validf = small.tile([NL, T], F32, tag="validf", name="validf")
            v.tensor_tensor(validf[:], vx[:], vy[:], op=ALU.mult)
            validf16 = small.tile([NL, T], F16, tag="validf16", name="validf16")
            v.tensor_copy(validf16[:], validf[:])
            denom = small.tile([NL, 1], F32, tag="denom", name="denom")
            v.reduce_sum(denom[:], validf[:], axis=mybir.AxisListType.X)

            pd = small.tile([NL, T, 2], F32, tag="pd", name="pd")
            v.tensor_tensor(pd[:], pred_t[:], targ_t[:], op=ALU.subtract)
            d2 = small.tile([NL, T], F32, tag="d2", name="d2")
            v._custom_dve(OP_SQ_ADD, out=d2[:].unsqueeze(2),
                          in0=pd[:, :, 0:1], in1=pd[:, :, 1:2], s0=0.0)
            dd = small.tile([NL, T], F32, tag="dd", name="dd")
            sc.activation(dd[:], d2[:], ACTF.Sqrt)
            dscr = small.tile([NL, T], F32, tag="dscr", name="dscr")
            dist_red = small.tile([NL, 1], F32, tag="dist_red", name="dist_red")
            v.scalar_tensor_tensor(dscr[:], dd[:], 1.0, validf[:],
                                   op0=ALU.mult, op1=ALU.mult,
                                   accum_out=dist_red[:])

            # fp16 copies used by the hot loop
            pred_x16 = small.tile([NL, T], F16, tag="pred_x16", name="pred_x16")
            v.tensor_copy(pred_x16[:].unsqueeze(2), pred_t[:, :, 0:1])
            pred_y16 = small.tile([NL, T], F16, tag="pred_y16", name="pred_y16")
            v.tensor_copy(pred_y16[:].unsqueeze(2), pred_t[:, :, 1:2])

            # ---- tiny per-(n,a,k) tensors, K-merged [NL, K, AL] -------------
            w2 = msz_t[:, :, 0:1]          # [NL, AL, 1]
            h2 = msz_t[:, :, 1:2]

            sw0 = small.tile([NL, AL], F16, tag="sw0", name="sw0")
            v.tensor_scalar(sw0[:].unsqueeze(2), w2, 0.5, w1h[:],
                            op0=ALU.mult, op1=ALU.add)
            sh0 = small.tile([NL, AL], F16, tag="sh0", name="sh0")
            v.tensor_scalar(sh0[:].unsqueeze(2), h2, 0.5, h1h[:],
                            op0=ALU.mult, op1=ALU.add)

            tau2 = small.tile([NL, K], F32, tag="tau2", name="tau2")
            for k in range(K):
                v.memset(tau2[:, k:k + 1], TAU[k] * 0.5)

            w2k = small.tile([NL, K, AL], F32, tag="w2k", name="w2k")
            h2k = small.tile([NL, K, AL], F32, tag="h2k", name="h2k")
            for k in range(K):
                v.tensor_scalar_add(w2k[:, k].unsqueeze(2), w2, TAU[k])
                v.tensor_scalar_add(h2k[:, k].unsqueeze(2), h2, TAU[k])

            def gh(src, egoscal, op, nm):
                """(min|max)(w1|h1, src) - tau_k/2 -> fp16 [NL, K, AL]."""
                t0 = small.tile([NL, K, AL], F32, tag="ghs", name=f"ghs_{nm}")
                v.tensor_scalar(t0[:], src[:], egoscal, None, op0=op)
                o = small.tile([NL, K, AL], F16, tag=nm, name=nm)
                v.tensor_tensor(o[:], t0[:],
                                tau2[:].unsqueeze(2).to_broadcast([NL, K, AL]),
                                op=ALU.subtract)
                return o

            GwA = gh(w2k, w1, ALU.min, "Gw")
            HwA = gh(w2k, w1, ALU.max, "Hw")
            GhA = gh(h2k, h1, ALU.min, "Gh")
            HhA = gh(h2k, h1, ALU.max, "Hh")

            area2 = small.tile([NL, K, AL], F32, tag="area2", name="area2")
            v.tensor_tensor(area2[:], w2k[:], h2k[:], op=ALU.mult)
            SA = small.tile([NL, K, AL], F16, tag="SA", name="SA")
            v.tensor_scalar(SA[:], area2[:], area1[:], EPS,
                            op0=ALU.add, op1=ALU.add)

            h2ke = small.tile([NL, K, AL], F32, tag="h2ke", name="h2ke")
            v.tensor_scalar_add(h2ke[:], h2k[:], EPS)
            rh2 = small.tile([NL, K, AL], F32, tag="rh2", name="rh2")
            v.reciprocal_approx_fast(rh2[:], h2ke[:])
            q2 = small.tile([NL, K, AL], F32, tag="q2", name="q2")
            v.tensor_tensor(q2[:], w2k[:], rh2[:], op=ALU.mult)
            at2 = small.tile([NL, K, AL], F32, tag="at2", name="at2")
            sc.activation(at2[:], q2[:], ACTF.Arctan)
            dat = small.tile([NL, K, AL], F32, tag="dat", name="dat")
            v.tensor_scalar(dat[:], at2[:], at1[:], None, op0=ALU.subtract)
            vk = small.tile([NL, K, AL], F32, tag="vk", name="vk")
            v._custom_dve(OP_SQ1, out=vk[:], in0=dat[:], s0=TWO_OVER_PI)
            V2A = small.tile([NL, K, AL], F16, tag="V2A", name="V2A")
            v._custom_dve(OP_SQ1, out=V2A[:], in0=vk[:], s0=1.0)
            PA = small.tile([NL, K, AL], F16, tag="PA", name="PA")
            v.tensor_scalar_add(PA[:], vk[:], 1.0 + EPS)

            # ---- main loop ---------------------------------------------------
            red_all = small.tile([NL, K * NCHUNK], F32, tag="red_all",
                                 name="red_all")

            def bck(tiny, ac):
                """[NL,K,AL] fp16 tiny -> [NL,K,T,ACH] bcast over t."""
                return (tiny[:, :, ac * ACH:(ac + 1) * ACH]
                        .unsqueeze(2).to_broadcast(FK))

            def bc1(tiny, ac):
                """[NL,AL] fp16 tiny -> [NL,T,ACH] bcast over t."""
                return (tiny[:, ac * ACH:(ac + 1) * ACH]
                        .unsqueeze(1).to_broadcast(FT))

            validf_b = validf16[:].unsqueeze(2).to_broadcast(FT)

            for ac in range(NCHUNK):
                csl = slice(ac * T * ACH, (ac + 1) * T * ACH)
                x2 = io.tile(FT, F16, tag="x2", name=f"x2_{ac}")
                nc.sync.dma_start(
                    x2[:], mamx[:, csl].rearrange("p (t a) -> p t a", a=ACH))
                y2 = io.tile(FT, F16, tag="y2", name=f"y2_{ac}")
                nc.sync.dma_start(
                    y2[:], mamy[:, csl].rearrange("p (t a) -> p t a", a=ACH))

                dx = kind.tile(FT, F16, tag="dx", name=f"dx{ac}")
                v.tensor_tensor(
                    dx[:], x2[:],
                    pred_x16[:].unsqueeze(2).to_broadcast(FT), op=ALU.subtract)
                dy = kind.tile(FT, F16, tag="dy", name=f"dy{ac}")
                v.tensor_tensor(
                    dy[:], y2[:],
                    pred_y16[:].unsqueeze(2).to_broadcast(FT), op=ALU.subtract)

                adx = kind.tile(FT, F16, tag="adx", name=f"adx{ac}")
                sc.activation(adx[:], dx[:], ACTF.Abs)
                ady = kind.tile(FT, F16, tag="ady", name=f"ady{ac}")
                sc.activation(ady[:], dy[:], ACTF.Abs)

                Uw = kind.tile(FT, F16, tag="Uw", name=f"Uw{ac}")
                v.tensor_tensor(Uw[:], bc1(sw0[:], ac), adx[:], op=ALU.subtract)
                Vw = kind.tile(FT, F16, tag="Vw", name=f"Vw{ac}")
                v.tensor_tensor(Vw[:], bc1(sw0[:], ac), adx[:], op=ALU.add)
                Uh = kind.tile(FT, F16, tag="Uh", name=f"Uh{ac}")
                v.tensor_tensor(Uh[:], bc1(sh0[:], ac), ady[:], op=ALU.subtract)
                Vh = kind.tile(FT, F16, tag="Vh", name=f"Vh{ac}")
                v.tensor_tensor(Vh[:], bc1(sh0[:], ac), ady[:], op=ALU.add)
                rho2 = kind.tile(FT, F16, tag="rho2", name=f"rho2{ac}")
                v._custom_dve(OP_SQ_ADD, out=rho2[:], in0=dx[:], in1=dy[:],
                              s0=0.0)

                def mgt(nm, nb=1):
                    return mg.tile(FK, F16, tag=nm, name=f"{nm}{ac}", bufs=nb)

                def bkk(kindep):
                    """[NL,T,ACH] k-indep -> [NL,K,T,ACH] bcast over k."""
                    return kindep[:].unsqueeze(1).to_broadcast(FK)

                twA = mgt("tw", 3)
                v.tensor_tensor(twA[:], bkk(Uw), bck(GwA[:], ac), op=ALU.min)
                thA = mgt("th", 3)
                v.tensor_tensor(thA[:], bkk(Uh), bck(GhA[:], ac), op=ALU.min)
                cwA = mgt("cw", 3)
                v.tensor_tensor(cwA[:], bkk(Vw), bck(HwA[:], ac), op=ALU.max)
                chA = mgt("ch", 3)
                v.tensor_tensor(chA[:], bkk(Vh), bck(HhA[:], ac), op=ALU.max)

                interA = twA          # in-place: inter overwrites tw
                c2A = cwA             # in-place: c2 overwrites cw
                for k in range(K):
                    v._custom_dve(OP_RELU_BIAS_MUL, out=interA[:, k],
                                  in0=twA[:, k], in1=thA[:, k], s0=TAU[k] * 0.5)
                    v._custom_dve(OP_SQB_ADD, out=c2A[:, k], in0=cwA[:, k],
                                  in1=chA[:, k], s0=TAU[k] * 0.5, s1=EPS)

                unionA = mgt("union", 3)
                g.tensor_tensor(unionA[:], bck(SA[:], ac), interA[:],
                                op=ALU.subtract)
                # in-place 1/x on ACT: ln then exp(-x)
                sc.activation(unionA[:], unionA[:], ACTF.Ln)
                sc.activation(unionA[:], unionA[:], ACTF.Exp, scale=-1.0)

                sc.activation(c2A[:], c2A[:], ACTF.Ln)
                sc.activation(c2A[:], c2A[:], ACTF.Exp, scale=-1.0)

                iouA = unionA         # iou = inter * r_union, onto union tile
                v.tensor_tensor(iouA[:], interA[:], unionA[:], op=ALU.mult)

                DkA = mgt("Dk", 3)
                g.tensor_tensor(DkA[:], bck(PA[:], ac), iouA[:],
                                op=ALU.subtract)
                sc.activation(DkA[:], DkA[:], ACTF.Ln)
                sc.activation(DkA[:], DkA[:], ACTF.Exp, scale=-1.0)

                avnA = DkA            # avn = v2 * r_D, onto Dk tile
                g.tensor_tensor(avnA[:], bck(V2A[:], ac), DkA[:], op=ALU.mult)

                RnA = c2A             # Rn = rho2 * r_c2, onto c2 tile
                v.tensor_tensor(RnA[:], rho2[:].unsqueeze(1).to_broadcast(FK),
                                c2A[:], op=ALU.mult)
                s1A = RnA             # s1 = iou - Rn, onto Rn tile
                v.tensor_tensor(s1A[:], iouA[:], RnA[:], op=ALU.subtract)
                s2A = s1A             # s2 = s1 - avn, onto s1 tile
                g.tensor_tensor(s2A[:], s1A[:], avnA[:], op=ALU.subtract)

                for k in range(K):
                    mscr = msc.tile(FT, F16, tag="mscr", name=f"ms{ac}_{k}")
                    ridx = ac * K + k
                    v.scalar_tensor_tensor(
                        mscr[:], s2A[:, k], WK[k], validf_b,
                        op0=ALU.mult, op1=ALU.mult,
                        accum_out=red_all[:, ridx:ridx + 1])

            # ---- finalize ----------------------------------------------------
            red_sum = small.tile([NL, 1], F32, tag="red_sum", name="red_sum")
            v.reduce_sum(red_sum[:], red_all[:], axis=mybir.AxisListType.X)
            out_t = small.tile([NL, 3], F32, tag="out_t", name="out_t")
            v.tensor_copy(out_t[:, 0:1], red_sum[:])
            v.tensor_copy(out_t[:, 1:2], denom[:])
            v.tensor_copy(out_t[:, 2:3], dist_red[:])
            nc.sync.dma_start(out[:], out_t[:])

    nc.compile()
    from concourse.bass_interp import get_hw_module
    nc.m = get_hw_module(nc.m)
    return nc


_CACHE = {}


def _get_nc(repeat=1):
    key = f"nc{repeat}"
    if key not in _CACHE:
        _CACHE[key] = _build_nc(repeat)
    return _CACHE[key]


def _shard_inputs(pred_motion, target_motion, ego_size, multiagent_size,
                  multiagents_motions):
    f32 = np.float32
    pred_motion = np.ascontiguousarray(pred_motion, f32)
    target_motion = np.ascontiguousarray(target_motion, f32)
    ego_size = np.ascontiguousarray(ego_size, f32)
    multiagent_size = np.ascontiguousarray(multiagent_size, f32)
    multiagents_motions = np.ascontiguousarray(multiagents_motions, f32)

    in_maps = []
    for c in range(8):
        nb, ah = c // 2, c % 2
        ns = slice(nb * NL, (nb + 1) * NL)
        asl = slice(ah * AL, (ah + 1) * AL)
        mm = multiagents_motions[ns, asl]          # (NL, AL, T, 2)
        # chunk-major (t, a) fp16 planes: [NL, NCHUNK, T, ACH] flattened
        planes = []
        for ci in range(2):                        # x then y
            p = mm[..., ci].transpose(0, 2, 1)     # (NL, T, AL)
            chunks = [p[:, :, ac * ACH:(ac + 1) * ACH].reshape(NL, -1)
                      for ac in range(NCHUNK)]
            planes.append(np.ascontiguousarray(
                np.concatenate(chunks, axis=1).astype(np.float16)))
        in_maps.append({
            "mamx": planes[0],
            "mamy": planes[1],
            "msz": np.ascontiguousarray(
                multiagent_size[ns, asl]).reshape(NL, AL * 2),
            "ego": ego_size[ns],
            "pred": pred_motion[ns].reshape(NL, T * 2),
            "targ": target_motion[ns].reshape(NL, T * 2),
        })
    return in_maps


def kernel(pred_motion, target_motion, ego_size, multiagent_size,
           multiagents_motions):
    nc = _get_nc()
    in_maps = _shard_inputs(pred_motion, target_motion, ego_size,
                            multiagent_size, multiagents_motions)

    from concourse.bass_utils import run_bass_kernel_spmd
    res = run_bass_kernel_spmd(nc, in_maps, core_ids=list(range(8)))
    outs = [r["out"] for r in res.results]          # each [NL, 3]

    # ---- host unshard: combine per-core partial sums ------------------------
    colsum = np.zeros(N, np.float64)
    denom = np.zeros(N, np.float64)
    dist_num = np.zeros(N, np.float64)
    for c in range(8):
        nb, ah = c // 2, c % 2
        ns = slice(nb * NL, (nb + 1) * NL)
        colsum[ns] += outs[c][:, 0].astype(np.float64)
        if ah == 0:
            denom[ns] = outs[c][:, 1].astype(np.float64)
            dist_num[ns] = outs[c][:, 2].astype(np.float64)

    col_loss = (colsum / denom).sum() / (N * A)
    dist_loss = dist_num.sum() / denom.sum()
    return np.float32(1.0 * dist_loss + 2.5 * col_loss)
